# revision 23
# baseline (speedup 1.0000x reference)
"""LSTM cell forward (nn_CellLSTM) on 8 trn2 NeuronCores — fp16 I/O.

Math (per reference):
    gates[g] = x @ ih4[g] + h_0 @ hh4[g] + ib4[g] + hb4[g]   for g in I,F,G,O
    c_1 = c_0 * sigmoid(F) + sigmoid(I) * tanh(G)
    h_1 = sigmoid(O) + tanh(c_1)
Outputs: (h_1, c_1, I_g, F_g, G_g, O_g), each [B, H] f32.

Sharding: pure data parallel over batch; each of 8 cores gets B/8 = 16384
contiguous rows; weights replicated. No collectives.

The problem is HBM-bandwidth bound (9 x B x H x 4B = 576 MB of f32 I/O).
With tolerance 2e-2 all device I/O moves to fp16 (host casts at the
boundary), halving DMA bytes; total fp16 error is ~1e-3 absmax-rel.

Per-core layout (supertile = 2048 batch rows, rpp = 16 subtiles,
b = s*2048 + p*16 + r so every regular DMA moves 128 x 4KB contiguous
lines):
  - x/h supertiles are loaded with the xbar DMA-transpose (fp16-only HW
    path) directly into SBUF as xT/hT [128=h, 2048=b] — no PE transposes,
    no PSUM round-trip.
  - subtile r: stationary operand = xT[:, r::16] (stride-16 column slice
    = batch rows p*16+r), moving = W [128, 512] in gate order I,F,O,G;
    x- and h- contributions accumulate in PSUM. 2 subtiles share one
    2-bank PSUM tile (3 tiles rotating) so the DVE bias-add runs once
    per group and PE/DVE ping-pong stays deep.
  - gates SBUF tile is GATE-MAJOR [p, (g r i)]: output DMAs get 4KB
    contiguous runs (256B runs would pay the <512B RMW 2x penalty),
    the sigmoid covers one contiguous I,F,O slab, and every combine
    reads flat [128, 1024] slabs (DVE 16-bit 2x mode).
  - DVE: bias-add fused with PSUM->SBUF move + all combines (per half-
    supertile); ACT: sigmoid(IFO), tanh(G), tanh(c_1).
  - queue discipline so no sequencer head waits on another engine's
    late data: SP (HWDGE) carries loads + the early-ready gate output
    DMAs; the late-ready h_1/c_1 DMAs go via GpSimd SWDGE.
  - per-tensor tile-pool tags with explicit bufs give real cross-
    supertile double buffering (untagged pool tiles share one ring).
  - the last supertile's output DMAs are issued per half (split_last)
    so the pipeline drain overlaps the final compute.
"""

import numpy as np

import concourse.bacc as bacc
import concourse.mybir as mybir
import concourse.tile as tile
from concourse import bass_utils

N_CORES = 8
B_FULL = 131072
H = 128
ROWS_PER_CORE = B_FULL // N_CORES

SUPER = 2048          # batch rows per supertile
RPP = SUPER // 128    # subtiles per supertile
GRP = 2               # subtiles per PSUM accumulation tile

F32 = mybir.dt.float32
F16 = mybir.dt.float16
AFT = mybir.ActivationFunctionType

OUT_NAMES = ("h_1", "c_1", "I_g", "F_g", "G_g", "O_g")
# W / bias column order (so sigmoid is one contiguous I,F,O slab)
GATE_ORDER = ("I_g", "F_g", "O_g", "G_g")
GATE_PERM = (0, 1, 3, 2)  # reference order I,F,G,O -> I,F,O,G


def build_nc(rows=ROWS_PER_CORE, super_rows=SUPER, repeat=1, dma_only=False,
             grp=GRP, h1_pool=False, c1_pool=None, t12_pool=False, dma_split=False,
             io_bufs=2, gates_bufs=3, act_bufs=2, out_bufs=3, pg_bufs=3,
             split_out=False, bias_pool=0, taper=0, split_last=True,
             load_split=0, warmup=True, warm_tags=False):
    rpp = super_rows // 128
    assert rows % super_rows == 0
    n_super = rows // super_rows
    n_grp = rpp // grp

    nc = bacc.Bacc("TRN2", target_bir_lowering=False)

    x = nc.dram_tensor("x", [rows, H], F16, kind="ExternalInput")
    h0 = nc.dram_tensor("h_0", [rows, H], F16, kind="ExternalInput")
    c0 = nc.dram_tensor("c_0", [rows, H], F16, kind="ExternalInput")
    wih = nc.dram_tensor("wih", [H, 4 * H], F16, kind="ExternalInput")
    whh = nc.dram_tensor("whh", [H, 4 * H], F16, kind="ExternalInput")
    bias = nc.dram_tensor("bias", [128, 4 * H], F32, kind="ExternalInput")
    outs = {
        name: nc.dram_tensor(name, [rows, H], F16, kind="ExternalOutput")
        for name in OUT_NAMES
    }

    # second HWDGE issue queue for the output DMAs
    odma = nc.scalar if dma_split else nc.sync

    # [n_super, 2048, 128] row-major views for the transpose loads
    def tview(t):
        return t.ap().rearrange("(s n) i -> s n i", n=super_rows)

    # [n_super, 128, rpp*128] interleaved views: b = s*super + p*rpp + r
    # -> per-partition lines are rpp*H contiguous fp16 elements (4KB)
    def iview(t):
        return t.ap().rearrange("(s p r) i -> s p (r i)", p=128, r=rpp)

    xv, hv, cv = tview(x), tview(h0), iview(c0)
    ov = {name: iview(t) for name, t in outs.items()}

    with tile.TileContext(nc) as tc:
        with (
            tc.tile_pool(name="const", bufs=1) as cpool,
            tc.tile_pool(name="io", bufs=3) as iop,
            tc.tile_pool(name="pgp", bufs=2, space="PSUM") as pgp,
            tc.tile_pool(name="gsb", bufs=3) as gsb,
            tc.tile_pool(name="actp", bufs=2) as actp,
            tc.tile_pool(name="resp", bufs=2) as resp,
            tc.tile_pool(name="outp", bufs=3) as outp,
        ):
            wih_t = cpool.tile([128, 4 * H], F16)
            whh_t = cpool.tile([128, 4 * H], F16)
            nc.sync.dma_start(wih_t[:], wih.ap())
            nc.sync.dma_start(whh_t[:], whh.ap())
            bias_t = cpool.tile([128, 4 * H], F32)
            nc.sync.dma_start(bias_t[:], bias.ap())
            if warmup:
                wrm = cpool.tile([128, 8], F16)
                nc.vector.memset(wrm[:], 0.0)
                wrm2 = cpool.tile([128, 8], F16)
                nc.scalar.activation(wrm2[:], wrm[:], AFT.Sigmoid)
                wrm3 = cpool.tile([128, 8], F16)
                nc.gpsimd.tensor_copy(wrm3[:], wrm[:])

            if dma_only:
                # timing probe: identical DMA traffic, zero compute
                for s in [s for _ in range(repeat) for s in range(n_super)]:
                    xT = iop.tile([128, super_rows], F16, tag="xT", bufs=io_bufs)
                    nc.sync.dma_start(xT[:], xv[s], transpose=True)
                    hT = iop.tile([128, super_rows], F16, tag="hT", bufs=io_bufs)
                    nc.sync.dma_start(hT[:], hv[s], transpose=True)
                    c_in = iop.tile([128, super_rows], F16, tag="c_in", bufs=io_bufs)
                    nc.sync.dma_start(c_in[:], cv[s])
                    zg = gsb.tile([128, rpp * 512], F16, tag="gates", bufs=gates_bufs)
                    zr = zg[:].rearrange("p (g ri) -> p g ri", g=4)
                    h1t = outp.tile([128, super_rows], F16, tag="h1t", bufs=out_bufs)
                    c1t = outp.tile([128, super_rows], F16, tag="c1t", bufs=out_bufs)
                    nc.vector.tensor_copy(h1t[:], c_in[:])
                    nc.vector.tensor_copy(c1t[:], c_in[:])
                    for g in range(4):
                        nc.vector.tensor_copy(zr[:, g, 0:super_rows], c_in[:])
                    odma.dma_start(ov["h_1"][s], h1t[:])
                    odma.dma_start(ov["c_1"][s], c1t[:])
                    for g, name in enumerate(GATE_ORDER):
                        odma.dma_start(ov[name][s], zr[:, g, :])
                nc.compile()
                return nc

            # variable-size segments: small first/last supertiles shorten
            # pipeline ramp and drain (the serial chain of the edge tiles)
            if taper and rows > 2 * super_rows:
                seg_sizes = ([taper] +
                             [super_rows] * ((rows - 2 * taper) // super_rows) +
                             [taper])
                assert sum(seg_sizes) == rows
            else:
                seg_sizes = [super_rows] * n_super
            segs = []
            off = 0
            for sz in seg_sizes:
                segs.append((off, sz))
                off += sz

            bv = bias_t[:].rearrange("p (g i) -> p g i", g=4)

            # queue discipline: no sequencer's head may wait on another
            # engine's late data. SP: input loads only. ACT: activations,
            # then the early-ready gate DMAs. Pool: the late-ready
            # h_1/c_1 DMAs (SWDGE) — all Pool-side deps.
            rep_segs = [t for _ in range(repeat) for t in segs]
            for seg_i, (off, sz) in enumerate(rep_segs):
                is_last = seg_i == len(rep_segs) - 1
                rpp_s = sz // 128
                n_grp = rpp_s // grp
                slab = rpp_s * H
                half = sz // 2
                xv_s = x.ap()[off:off + sz, :]
                hv_s = h0.ap()[off:off + sz, :]
                cv_s = c0.ap()[off:off + sz, :].rearrange(
                    "(p r) i -> p (r i)", p=128)
                ov_s = {
                    name: t.ap()[off:off + sz, :].rearrange(
                        "(p r) i -> p (r i)", p=128)
                    for name, t in outs.items()
                }

                h_eng = nc.scalar if load_split >= 1 else nc.sync
                c_eng = nc.scalar if load_split >= 2 else nc.sync
                wtag = "w" if (warm_tags and seg_i < 2) else ""
                xT = iop.tile([128, sz], F16, tag="xT" + wtag,
                              bufs=2 if wtag else io_bufs,
                              padded_shape=[128, super_rows])
                nc.sync.dma_start(xT[:], xv_s, transpose=True)
                hT = iop.tile([128, sz], F16, tag="hT" + wtag,
                              bufs=2 if wtag else io_bufs,
                              padded_shape=[128, super_rows])
                h_eng.dma_start(hT[:], hv_s, transpose=True)
                c_in = iop.tile([128, sz], F16, tag="c_in" + wtag,
                                bufs=2 if wtag else io_bufs,
                                padded_shape=[128, super_rows])
                c_eng.dma_start(c_in[:], cv_s)

                # gate-major: gates[p, g*rpp_s*H + r*H + i]
                gates = gsb.tile([128, rpp_s * 512], F16, tag="gates",
                                 bufs=gates_bufs,
                                 padded_shape=[128, RPP * 512])
                act = actp.tile([128, rpp_s * 512], F16, tag="act",
                                bufs=act_bufs, padded_shape=[128, RPP * 512])
                c1t = outp.tile([128, sz], F16, tag="c1t", bufs=out_bufs,
                                padded_shape=[128, super_rows])
                h1t = outp.tile([128, sz], F16, tag="h1t", bufs=out_bufs,
                                padded_shape=[128, super_rows])

                # [p, r, q] with column b = q*rpp_s + r
                xTv = xT[:].rearrange("p (q r) -> p r q", r=rpp_s)
                hTv = hT[:].rearrange("p (q r) -> p r q", r=rpp_s)
                # bias-add target: [p, g, t, (q i)]
                gview = gates[:].rearrange("p (g t qi) -> p g t qi",
                                           g=4, t=n_grp)

                for t in range(n_grp):
                    pg = pgp.tile([128, grp * 512], F32, tag="pg", bufs=pg_bufs)
                    for q in range(grp):
                        r = t * grp + q
                        ps = slice(q * 512, (q + 1) * 512)
                        nc.tensor.matmul(pg[:, ps], xTv[:, r, :],
                                         wih_t[:], start=True, stop=False)
                        nc.tensor.matmul(pg[:, ps], hTv[:, r, :],
                                         whh_t[:], start=False, stop=True)
                    # PSUM->SBUF move fused with bias add; gate-major out
                    pgv = pg[:].rearrange("p (q g i) -> p g q i", g=4, i=H)
                    stride = n_grp // bias_pool if bias_pool else 0
                    if bias_pool and t % stride == stride - 1:
                        b_eng = nc.gpsimd
                    else:
                        b_eng = nc.vector
                    b_eng.tensor_add(
                        gview[:, :, t, :], pgv,
                        bv.unsqueeze(2).broadcast_to([128, 4, grp, H]),
                    )

                g4 = gates[:].rearrange("p (g ri) -> p g ri", g=4)
                a4 = act[:].rearrange("p (g ri) -> p g ri", g=4)
                hss = [slice(hf * half, (hf + 1) * half) for hf in range(2)]
                for hs in hss:
                    nc.scalar.activation(a4[:, 0:3, hs], g4[:, 0:3, hs],
                                         AFT.Sigmoid)
                    nc.scalar.activation(a4[:, 3, hs], g4[:, 3, hs], AFT.Tanh)

                t12_eng = nc.gpsimd if t12_pool else nc.vector
                t1s, t2s = [], []
                for hf, hs in enumerate(hss):
                    sigI = a4[:, 0, hs]
                    sigF = a4[:, 1, hs]
                    tanG = a4[:, 3, hs]
                    t1 = resp.tile([128, half], F16, tag="t1", bufs=3)
                    t12_eng.tensor_mul(t1[:], c_in[:, hs], sigF)  # c0*sigF
                    t2 = resp.tile([128, half], F16, tag="t2", bufs=3)
                    t12_eng.tensor_mul(t2[:], sigI, tanG)       # sigI*tanG
                    t1s.append(t1)
                    t2s.append(t2)

                if c1_pool is None:
                    _c1_pool = h1_pool
                else:
                    _c1_pool = c1_pool
                c1_eng = nc.gpsimd if _c1_pool else nc.vector
                h1_eng = nc.gpsimd if h1_pool else nc.vector
                th1s = []
                for hf, hs in enumerate(hss):
                    c1_eng.tensor_add(c1t[:, hs], t1s[hf][:], t2s[hf][:])
                    th1 = resp.tile([128, half], F16, tag="th1", bufs=3)
                    th1s.append(th1)
                for hf, hs in enumerate(hss):
                    nc.scalar.activation(th1s[hf][:], c1t[:, hs], AFT.Tanh)
                for hf, hs in enumerate(hss):
                    sigO = a4[:, 2, hs]
                    h1_eng.tensor_add(h1t[:, hs], sigO, th1s[hf][:])

                gv = gates[:].rearrange("p (g ri) -> p g ri", g=4)
                if split_out or (split_last and is_last):
                    for hf, hs in enumerate(hss):
                        for g, name in enumerate(GATE_ORDER):
                            odma.dma_start(ov_s[name][:, hs], gv[:, g, hs])
                        nc.gpsimd.dma_start(ov_s["c_1"][:, hs], c1t[:, hs])
                        nc.gpsimd.dma_start(ov_s["h_1"][:, hs], h1t[:, hs])
                else:
                    for g, name in enumerate(GATE_ORDER):
                        odma.dma_start(ov_s[name], gv[:, g, :])
                    nc.gpsimd.dma_start(ov_s["c_1"], c1t[:])
                    nc.gpsimd.dma_start(ov_s["h_1"], h1t[:])

    nc.compile()
    return nc


_NC_CACHE = {}


def _get_nc(rows=ROWS_PER_CORE):
    if rows not in _NC_CACHE:
        _NC_CACHE[rows] = build_nc(rows)
    return _NC_CACHE[rows]


def _prep_host_inputs(x, h_0, c_0, ih, hh, ib, hb):
    """Cast activations to fp16 and pre-layout the replicated params."""
    x16 = np.asarray(x, dtype=np.float16)
    h16 = np.asarray(h_0, dtype=np.float16)
    c16 = np.asarray(c_0, dtype=np.float16)
    ih = np.asarray(ih, dtype=np.float32)
    hh = np.asarray(hh, dtype=np.float32)
    ib = np.asarray(ib, dtype=np.float32)
    hb = np.asarray(hb, dtype=np.float32)
    perm = list(GATE_PERM)
    # W[h, g*128+i] = ih[perm[g]*128+h, i]
    wih = np.ascontiguousarray(
        ih.reshape(4, H, H)[perm].transpose(1, 0, 2).reshape(H, 4 * H)
    ).astype(np.float16)
    whh = np.ascontiguousarray(
        hh.reshape(4, H, H)[perm].transpose(1, 0, 2).reshape(H, 4 * H)
    ).astype(np.float16)
    b = (ib + hb).reshape(4, H)[perm].reshape(1, 4 * H)
    bias = np.ascontiguousarray(np.broadcast_to(b, (128, 4 * H)),
                                dtype=np.float32)
    return x16, h16, c16, wih, whh, bias


def make_in_maps(x, h_0, c_0, ih, hh, ib, hb):
    x16, h16, c16, wih, whh, bias = _prep_host_inputs(
        x, h_0, c_0, ih, hh, ib, hb)
    in_maps = []
    for i in range(N_CORES):
        sl = slice(i * ROWS_PER_CORE, (i + 1) * ROWS_PER_CORE)
        in_maps.append(
            dict(
                x=np.ascontiguousarray(x16[sl]),
                h_0=np.ascontiguousarray(h16[sl]),
                c_0=np.ascontiguousarray(c16[sl]),
                wih=wih,
                whh=whh,
                bias=bias,
            )
        )
    return in_maps


def run_sharded(x, h_0, c_0, ih, hh, ib, hb, **spmd_kwargs):
    in_maps = make_in_maps(x, h_0, c_0, ih, hh, ib, hb)
    nc = _get_nc()
    res = bass_utils.run_bass_kernel_spmd(
        nc, in_maps, core_ids=list(range(N_CORES)), **spmd_kwargs
    )
    outs = res.results
    full = tuple(
        np.concatenate([outs[i][name] for i in range(N_CORES)], axis=0)
        .astype(np.float32)
        for name in OUT_NAMES
    )
    return full, res


def kernel(x, h_0, c_0, ih, hh, ib, hb):
    full, _ = run_sharded(x, h_0, c_0, ih, hh, ib, hb)
    return full


# revision 26
# speedup vs baseline: 1.1747x; 1.1747x over previous
"""LSTM cell forward (nn_CellLSTM) on 8 trn2 NeuronCores — fp16 I/O.

Math (per reference):
    gates[g] = x @ ih4[g] + h_0 @ hh4[g] + ib4[g] + hb4[g]   for g in I,F,G,O
    c_1 = c_0 * sigmoid(F) + sigmoid(I) * tanh(G)
    h_1 = sigmoid(O) + tanh(c_1)
Outputs: (h_1, c_1, I_g, F_g, G_g, O_g), each [B, H] f32.

Sharding: pure data parallel over batch; each of 8 cores gets B/8 = 16384
contiguous rows; weights replicated. No collectives.

The problem is HBM-bandwidth bound (9 x B x H x 4B = 576 MB of f32 I/O).
With tolerance 2e-2 all device I/O moves to fp16 (host casts at the
boundary), halving DMA bytes; total fp16 error is ~1e-3 absmax-rel.

Per-core layout (supertile = 2048 batch rows, rpp = 16 subtiles,
b = s*2048 + p*16 + r so every regular DMA moves 128 x 4KB contiguous
lines):
  - x/h supertiles are loaded with the xbar DMA-transpose (fp16-only HW
    path) directly into SBUF as xT/hT [128=h, 2048=b] — no PE transposes,
    no PSUM round-trip.
  - subtile r: stationary operand = xT[:, r::16] (stride-16 column slice
    = batch rows p*16+r), moving = W [128, 512] in gate order I,F,O,G;
    x- and h- contributions accumulate in PSUM. 2 subtiles share one
    2-bank PSUM tile (3 tiles rotating) so the DVE bias-add runs once
    per group and PE/DVE ping-pong stays deep.
  - gates SBUF tile is GATE-MAJOR [p, (g r i)]: output DMAs get 4KB
    contiguous runs (256B runs would pay the <512B RMW 2x penalty),
    the sigmoid covers one contiguous I,F,O slab, and every combine
    reads flat [128, 1024] slabs (DVE 16-bit 2x mode).
  - DVE: bias-add fused with PSUM->SBUF move + all combines (per half-
    supertile); ACT: sigmoid(IFO), tanh(G), tanh(c_1).
  - queue discipline so no sequencer head waits on another engine's
    late data: SP (HWDGE) carries loads + the early-ready gate output
    DMAs; the late-ready h_1/c_1 DMAs go via GpSimd SWDGE.
  - per-tensor tile-pool tags with explicit bufs give real cross-
    supertile double buffering (untagged pool tiles share one ring).
  - the last supertile's output DMAs are issued per half (split_last)
    so the pipeline drain overlaps the final compute.
"""

import numpy as np

import concourse.bacc as bacc
import concourse.mybir as mybir
import concourse.tile as tile
from concourse import bass_utils

N_CORES = 8
B_FULL = 131072
H = 128
ROWS_PER_CORE = B_FULL // N_CORES

SUPER = 2048          # batch rows per supertile
RPP = SUPER // 128    # subtiles per supertile
GRP = 2               # subtiles per PSUM accumulation tile

F32 = mybir.dt.float32
F16 = mybir.dt.float16
AFT = mybir.ActivationFunctionType

OUT_NAMES = ("h_1", "c_1", "I_g", "F_g", "G_g", "O_g")
# W / bias column order (so sigmoid is one contiguous I,F,O slab)
GATE_ORDER = ("I_g", "F_g", "O_g", "G_g")
GATE_PERM = (0, 1, 3, 2)  # reference order I,F,G,O -> I,F,O,G


def build_nc(rows=ROWS_PER_CORE, super_rows=SUPER, repeat=1, dma_only=False,
             grp=GRP, h1_pool=False, c1_pool=None, t12_pool=False, dma_split=False,
             io_bufs=2, gates_bufs=3, act_bufs=2, out_bufs=3, pg_bufs=3,
             split_out=False, bias_pool=0, taper=0, split_last=True,
             load_split=0, warmup=True, warm_tags=False, const_split=False,
             ramp_split=False, n_split=2):
    rpp = super_rows // 128
    assert rows % super_rows == 0
    n_super = rows // super_rows
    n_grp = rpp // grp

    nc = bacc.Bacc("TRN2", target_bir_lowering=False)

    x = nc.dram_tensor("x", [rows, H], F16, kind="ExternalInput")
    h0 = nc.dram_tensor("h_0", [rows, H], F16, kind="ExternalInput")
    c0 = nc.dram_tensor("c_0", [rows, H], F16, kind="ExternalInput")
    wih = nc.dram_tensor("wih", [H, 4 * H], F16, kind="ExternalInput")
    whh = nc.dram_tensor("whh", [H, 4 * H], F16, kind="ExternalInput")
    bias = nc.dram_tensor("bias", [128, 4 * H], F32, kind="ExternalInput")
    outs = {
        name: nc.dram_tensor(name, [rows, H], F16, kind="ExternalOutput")
        for name in OUT_NAMES
    }

    # second HWDGE issue queue for the output DMAs
    odma = nc.scalar if dma_split else nc.sync

    # [n_super, 2048, 128] row-major views for the transpose loads
    def tview(t):
        return t.ap().rearrange("(s n) i -> s n i", n=super_rows)

    # [n_super, 128, rpp*128] interleaved views: b = s*super + p*rpp + r
    # -> per-partition lines are rpp*H contiguous fp16 elements (4KB)
    def iview(t):
        return t.ap().rearrange("(s p r) i -> s p (r i)", p=128, r=rpp)

    xv, hv, cv = tview(x), tview(h0), iview(c0)
    ov = {name: iview(t) for name, t in outs.items()}

    with tile.TileContext(nc) as tc:
        with (
            tc.tile_pool(name="const", bufs=1) as cpool,
            tc.tile_pool(name="io", bufs=3) as iop,
            tc.tile_pool(name="pgp", bufs=2, space="PSUM") as pgp,
            tc.tile_pool(name="gsb", bufs=3) as gsb,
            tc.tile_pool(name="actp", bufs=2) as actp,
            tc.tile_pool(name="resp", bufs=2) as resp,
            tc.tile_pool(name="outp", bufs=3) as outp,
        ):
            wih_t = cpool.tile([128, 4 * H], F16)
            whh_t = cpool.tile([128, 4 * H], F16)
            cdma = nc.scalar if const_split else nc.sync
            cdma.dma_start(wih_t[:], wih.ap())
            cdma.dma_start(whh_t[:], whh.ap())
            bias_t = cpool.tile([128, 4 * H], F32)
            cdma.dma_start(bias_t[:], bias.ap())
            if warmup:
                wrm = cpool.tile([128, 8], F16)
                nc.vector.memset(wrm[:], 0.0)
                wrm2 = cpool.tile([128, 8], F16)
                nc.scalar.activation(wrm2[:], wrm[:], AFT.Sigmoid)
                wrm3 = cpool.tile([128, 8], F16)
                nc.gpsimd.tensor_copy(wrm3[:], wrm[:])

            if dma_only:
                # timing probe: identical DMA traffic, zero compute
                for s in [s for _ in range(repeat) for s in range(n_super)]:
                    xT = iop.tile([128, super_rows], F16, tag="xT", bufs=io_bufs)
                    nc.sync.dma_start(xT[:], xv[s], transpose=True)
                    hT = iop.tile([128, super_rows], F16, tag="hT", bufs=io_bufs)
                    nc.sync.dma_start(hT[:], hv[s], transpose=True)
                    c_in = iop.tile([128, super_rows], F16, tag="c_in", bufs=io_bufs)
                    nc.sync.dma_start(c_in[:], cv[s])
                    zg = gsb.tile([128, rpp * 512], F16, tag="gates", bufs=gates_bufs)
                    zr = zg[:].rearrange("p (g ri) -> p g ri", g=4)
                    h1t = outp.tile([128, super_rows], F16, tag="h1t", bufs=out_bufs)
                    c1t = outp.tile([128, super_rows], F16, tag="c1t", bufs=out_bufs)
                    nc.vector.tensor_copy(h1t[:], c_in[:])
                    nc.vector.tensor_copy(c1t[:], c_in[:])
                    for g in range(4):
                        nc.vector.tensor_copy(zr[:, g, 0:super_rows], c_in[:])
                    odma.dma_start(ov["h_1"][s], h1t[:])
                    odma.dma_start(ov["c_1"][s], c1t[:])
                    for g, name in enumerate(GATE_ORDER):
                        odma.dma_start(ov[name][s], zr[:, g, :])
                nc.compile()
                return nc

            # variable-size segments: small first/last supertiles shorten
            # pipeline ramp and drain (the serial chain of the edge tiles)
            if taper and rows > 2 * super_rows:
                seg_sizes = ([taper] +
                             [super_rows] * ((rows - 2 * taper) // super_rows) +
                             [taper])
                assert sum(seg_sizes) == rows
            else:
                seg_sizes = [super_rows] * n_super
            segs = []
            off = 0
            for sz in seg_sizes:
                segs.append((off, sz))
                off += sz

            bv = bias_t[:].rearrange("p (g i) -> p g i", g=4)

            # queue discipline: no sequencer's head may wait on another
            # engine's late data. SP: input loads only. ACT: activations,
            # then the early-ready gate DMAs. Pool: the late-ready
            # h_1/c_1 DMAs (SWDGE) — all Pool-side deps.
            rep_segs = [t for _ in range(repeat) for t in segs]
            for seg_i, (off, sz) in enumerate(rep_segs):
                is_last = seg_i == len(rep_segs) - 1
                rpp_s = sz // 128
                n_grp = rpp_s // grp
                slab = rpp_s * H
                half = sz // n_split
                xv_s = x.ap()[off:off + sz, :]
                hv_s = h0.ap()[off:off + sz, :]
                cv_s = c0.ap()[off:off + sz, :].rearrange(
                    "(p r) i -> p (r i)", p=128)
                ov_s = {
                    name: t.ap()[off:off + sz, :].rearrange(
                        "(p r) i -> p (r i)", p=128)
                    for name, t in outs.items()
                }

                h_eng = nc.scalar if load_split >= 1 else nc.sync
                c_eng = nc.scalar if load_split >= 2 else nc.sync
                if ramp_split and seg_i < 2:
                    h_eng = nc.scalar
                    c_eng = nc.gpsimd
                wtag = "w" if (warm_tags and seg_i < 2) else ""
                xT = iop.tile([128, sz], F16, tag="xT" + wtag,
                              bufs=2 if wtag else io_bufs,
                              padded_shape=[128, super_rows])
                nc.sync.dma_start(xT[:], xv_s, transpose=True)
                hT = iop.tile([128, sz], F16, tag="hT" + wtag,
                              bufs=2 if wtag else io_bufs,
                              padded_shape=[128, super_rows])
                h_eng.dma_start(hT[:], hv_s, transpose=True)
                c_in = iop.tile([128, sz], F16, tag="c_in" + wtag,
                                bufs=2 if wtag else io_bufs,
                                padded_shape=[128, super_rows])
                c_eng.dma_start(c_in[:], cv_s)

                # gate-major: gates[p, g*rpp_s*H + r*H + i]
                gates = gsb.tile([128, rpp_s * 512], F16, tag="gates",
                                 bufs=gates_bufs,
                                 padded_shape=[128, RPP * 512])
                act = actp.tile([128, rpp_s * 512], F16, tag="act",
                                bufs=act_bufs, padded_shape=[128, RPP * 512])
                c1t = outp.tile([128, sz], F16, tag="c1t", bufs=out_bufs,
                                padded_shape=[128, super_rows])
                h1t = outp.tile([128, sz], F16, tag="h1t", bufs=out_bufs,
                                padded_shape=[128, super_rows])

                # [p, r, q] with column b = q*rpp_s + r
                xTv = xT[:].rearrange("p (q r) -> p r q", r=rpp_s)
                hTv = hT[:].rearrange("p (q r) -> p r q", r=rpp_s)
                # bias-add target: [p, g, t, (q i)]
                gview = gates[:].rearrange("p (g t qi) -> p g t qi",
                                           g=4, t=n_grp)

                for t in range(n_grp):
                    pg = pgp.tile([128, grp * 512], F32, tag="pg", bufs=pg_bufs)
                    for q in range(grp):
                        r = t * grp + q
                        ps = slice(q * 512, (q + 1) * 512)
                        nc.tensor.matmul(pg[:, ps], xTv[:, r, :],
                                         wih_t[:], start=True, stop=False)
                        nc.tensor.matmul(pg[:, ps], hTv[:, r, :],
                                         whh_t[:], start=False, stop=True)
                    # PSUM->SBUF move fused with bias add; gate-major out
                    pgv = pg[:].rearrange("p (q g i) -> p g q i", g=4, i=H)
                    stride = n_grp // bias_pool if bias_pool else 0
                    if bias_pool and t % stride == stride - 1:
                        b_eng = nc.gpsimd
                    else:
                        b_eng = nc.vector
                    b_eng.tensor_add(
                        gview[:, :, t, :], pgv,
                        bv.unsqueeze(2).broadcast_to([128, 4, grp, H]),
                    )

                g4 = gates[:].rearrange("p (g ri) -> p g ri", g=4)
                a4 = act[:].rearrange("p (g ri) -> p g ri", g=4)
                hss = [slice(hf * half, (hf + 1) * half)
                       for hf in range(n_split)]
                for hs in hss:
                    nc.scalar.activation(a4[:, 0:3, hs], g4[:, 0:3, hs],
                                         AFT.Sigmoid)
                    nc.scalar.activation(a4[:, 3, hs], g4[:, 3, hs], AFT.Tanh)

                t12_eng = nc.gpsimd if t12_pool else nc.vector
                t1s, t2s = [], []
                for hf, hs in enumerate(hss):
                    sigI = a4[:, 0, hs]
                    sigF = a4[:, 1, hs]
                    tanG = a4[:, 3, hs]
                    t1 = resp.tile([128, half], F16, tag="t1", bufs=3)
                    t12_eng.tensor_mul(t1[:], c_in[:, hs], sigF)  # c0*sigF
                    t2 = resp.tile([128, half], F16, tag="t2", bufs=3)
                    t12_eng.tensor_mul(t2[:], sigI, tanG)       # sigI*tanG
                    t1s.append(t1)
                    t2s.append(t2)

                if c1_pool is None:
                    _c1_pool = h1_pool
                else:
                    _c1_pool = c1_pool
                c1_eng = nc.gpsimd if _c1_pool else nc.vector
                h1_eng = nc.gpsimd if h1_pool else nc.vector
                th1s = []
                for hf, hs in enumerate(hss):
                    c1_eng.tensor_add(c1t[:, hs], t1s[hf][:], t2s[hf][:])
                    th1 = resp.tile([128, half], F16, tag="th1", bufs=3)
                    th1s.append(th1)
                for hf, hs in enumerate(hss):
                    nc.scalar.activation(th1s[hf][:], c1t[:, hs], AFT.Tanh)
                for hf, hs in enumerate(hss):
                    sigO = a4[:, 2, hs]
                    h1_eng.tensor_add(h1t[:, hs], sigO, th1s[hf][:])

                gv = gates[:].rearrange("p (g ri) -> p g ri", g=4)
                if split_out or (split_last and is_last):
                    for hf, hs in enumerate(hss):
                        for g, name in enumerate(GATE_ORDER):
                            odma.dma_start(ov_s[name][:, hs], gv[:, g, hs])
                        nc.gpsimd.dma_start(ov_s["c_1"][:, hs], c1t[:, hs])
                        nc.gpsimd.dma_start(ov_s["h_1"][:, hs], h1t[:, hs])
                else:
                    for g, name in enumerate(GATE_ORDER):
                        odma.dma_start(ov_s[name], gv[:, g, :])
                    nc.gpsimd.dma_start(ov_s["c_1"], c1t[:])
                    nc.gpsimd.dma_start(ov_s["h_1"], h1t[:])

    nc.compile()
    return nc


_NC_CACHE = {}


def _get_nc(rows=ROWS_PER_CORE):
    if rows not in _NC_CACHE:
        _NC_CACHE[rows] = build_nc(rows)
    return _NC_CACHE[rows]


def _prep_host_inputs(x, h_0, c_0, ih, hh, ib, hb):
    """Cast activations to fp16 and pre-layout the replicated params."""
    x16 = np.asarray(x, dtype=np.float16)
    h16 = np.asarray(h_0, dtype=np.float16)
    c16 = np.asarray(c_0, dtype=np.float16)
    ih = np.asarray(ih, dtype=np.float32)
    hh = np.asarray(hh, dtype=np.float32)
    ib = np.asarray(ib, dtype=np.float32)
    hb = np.asarray(hb, dtype=np.float32)
    perm = list(GATE_PERM)
    # W[h, g*128+i] = ih[perm[g]*128+h, i]
    wih = np.ascontiguousarray(
        ih.reshape(4, H, H)[perm].transpose(1, 0, 2).reshape(H, 4 * H)
    ).astype(np.float16)
    whh = np.ascontiguousarray(
        hh.reshape(4, H, H)[perm].transpose(1, 0, 2).reshape(H, 4 * H)
    ).astype(np.float16)
    b = (ib + hb).reshape(4, H)[perm].reshape(1, 4 * H)
    bias = np.ascontiguousarray(np.broadcast_to(b, (128, 4 * H)),
                                dtype=np.float32)
    return x16, h16, c16, wih, whh, bias


def make_in_maps(x, h_0, c_0, ih, hh, ib, hb):
    x16, h16, c16, wih, whh, bias = _prep_host_inputs(
        x, h_0, c_0, ih, hh, ib, hb)
    in_maps = []
    for i in range(N_CORES):
        sl = slice(i * ROWS_PER_CORE, (i + 1) * ROWS_PER_CORE)
        in_maps.append(
            dict(
                x=np.ascontiguousarray(x16[sl]),
                h_0=np.ascontiguousarray(h16[sl]),
                c_0=np.ascontiguousarray(c16[sl]),
                wih=wih,
                whh=whh,
                bias=bias,
            )
        )
    return in_maps


def run_sharded(x, h_0, c_0, ih, hh, ib, hb, **spmd_kwargs):
    in_maps = make_in_maps(x, h_0, c_0, ih, hh, ib, hb)
    nc = _get_nc()
    res = bass_utils.run_bass_kernel_spmd(
        nc, in_maps, core_ids=list(range(N_CORES)), **spmd_kwargs
    )
    outs = res.results
    full = tuple(
        np.concatenate([outs[i][name] for i in range(N_CORES)], axis=0)
        .astype(np.float32)
        for name in OUT_NAMES
    )
    return full, res


def kernel(x, h_0, c_0, ih, hh, ib, hb):
    full, _ = run_sharded(x, h_0, c_0, ih, hh, ib, hb)
    return full
